# revision 9
# baseline (speedup 1.0000x reference)
"""CommonNeighborsPredictor kernel for 8 Trainium2 NeuronCores.

Math (see reference):
    deg = adj.sum(-1) + 1e-6
    x   = emb + (adj @ emb) / deg[:, None]
    xn  = x / max(||x||_2, 1e-8)                            # row-normalize
    w_e = sum_c adj[src_e, c] * adj[dst_e, c] * (xn[src_e]@xn[c]) * (xn[dst_e]@xn[c])
    out = sigmoid(w)

Distribution (2 SPMD launches, no collectives):

  Stage 1 (node-major): shard nodes 8 ways; core k computes xn for its 1250
    nodes.  The k-loop streams adjT tiles [128k, 1280m] (one wide DMA each)
    and emb k-slices [128k, 256]; the PE runs 10 matmuls per k-tile with the
    adjT slice as the stationary operand, accumulating y = adj@emb in
    node-major PSUM ([128 nodes x 256 dims], two blocks packed per bank).
    Degrees accumulate on DVE (exact 0/1 sums in bf16) and are reduced
    per-node with tiny N=1 transpose-matmuls.  The epilogue uses the
    scale-invariance of cosine: x' = deg*emb + y (no division), per-node
    scalars live in [128,1] columns (fast DVE reciprocal + ACT sqrt), and
    scale application is a 4x-mode DVE tensor_scalar.  Host transposes the
    node-major xn shards into xnT.

  Stage 2 (candidate-major): shard query edges 8 ways (512 each).  The host
    lays out per-edge adjacency tables TRANSPOSED and pre-tiled
    (est[p, 512*ct + e] = adj[src_e, 128*ct+p]) so the kernel does plain
    sequential DMA - no indirect gathers, no gpsimd descriptor storms.  The
    src*dst mask product cn = min(aS, aD) is computed INLINE in the DMA
    (CCE min accumulate on the SWDGE path) - no compute engine touches it.
    Per candidate tile [128c x 512e]: PE matmuls produce cosL/cosR against
    resident xnT slices (stationary) and ut|vt (moving), ACT copies the
    PSUM to bf16 SBUF, DVE does the two mask/cos products at 2x bf16 rate,
    and a ones-vector matmul accumulates the candidate-dim reduction across
    all 79 tiles into a single [1, 512] PSUM row.  Sigmoid on ACT.

dtypes: adjacency and matmul operands bf16 (adjacency 0/1 exact; emb/xn
rounding contributes ~1e-4 output error vs the fp32 reference).  PSUM and
per-node scalars fp32.
"""

import numpy as np

import concourse.bass as bass
import concourse.bacc as bacc
import concourse.mybir as mybir
import concourse.tile as tile
from concourse import bass_utils

F32 = mybir.dt.float32
BF16 = mybir.dt.bfloat16
FP8 = mybir.dt.float8e4
AF = mybir.ActivationFunctionType
OP = mybir.AluOpType
NP_BF16 = mybir.dt.np(BF16)

N, D, Q, NC = 10000, 256, 4096, 8
KT = 79                  # contraction tiles over source nodes (N padded)
KP = KT * 128            # 10112
MSH = N // NC            # 1250 nodes per core
MB = 10                  # node blocks per core
MSH_P = MB * 128         # 1280 (padded shard)
QL = Q // NC             # 512 edges per core
CT = 79                  # candidate tiles in stage 2
NP_PAD = CT * 128        # 10112
CH = 8                   # candidate tiles per mask DMA chunk
NCH = (CT + CH - 1) // CH


def _chunk_plan(total, first, step):
    out, base = [], 0
    n = first
    while base < total:
        n = min(n, total - base)
        out.append((base, n))
        base += n
        n = step
    return out


def build_stage1(mm_dt=BF16):
    """Per-core: xn [1280, 256] node-major from adjT shard + emb."""
    b = bacc.Bacc("TRN2", target_bir_lowering=False, debug=False, num_devices=NC)
    adjT = b.dram_tensor("adjT", [128, KT * MSH_P], mm_dt, kind="ExternalInput")
    embx = b.dram_tensor("embx", [128, KT * D], mm_dt, kind="ExternalInput")
    embn = b.dram_tensor("embn", [128, MB * D], mm_dt, kind="ExternalInput")
    xn = b.dram_tensor("xn", [MSH_P, D], mm_dt, kind="ExternalOutput")

    NDEG = 2
    ACHUNKS = _chunk_plan(KT, 2, 8)
    with tile.TileContext(b) as tc:
        with (
            tc.tile_pool(name="const", bufs=1) as cpool,
            tc.tile_pool(name="adjs", bufs=3) as apool,
            tc.tile_pool(name="work", bufs=3) as wpool,
            tc.tile_pool(name="py", bufs=1, space="PSUM") as ypool,
            tc.tile_pool(name="pd", bufs=1, space="PSUM") as dpool,
        ):
            ones_col = cpool.tile([128, 1], mm_dt)
            b.vector.memset(ones_col[:, :1], 1.0)
            deg_p = [
                cpool.tile([128, MSH_P], mm_dt, tag=f"degp{c}", name=f"degp{c}")
                for c in range(NDEG)
            ]
            ps_y = [
                ypool.tile([128, 2 * D], F32, tag=f"py{h}", name=f"py{h}")
                for h in range(MB // 2)
            ]
            deg_ps = dpool.tile([128, MB], F32, tag="degps")

            # resident emb (k-major) loaded in 3 big chunks; first covers the
            # first k-tiles so the PE can start immediately
            embx_sb = cpool.tile([128, KT * D], mm_dt, tag="embx")
            ECH = [(0, 2048), (2048, 6144), (8192, KT * D - 8192)]

            def load_embx(ci):
                c0, cw = ECH[ci]
                b.sync.dma_start(
                    out=embx_sb[:, c0 : c0 + cw], in_=embx.ap()[:, c0 : c0 + cw]
                )

            embn_sb = cpool.tile([128, MB * D], mm_dt, tag="embn")
            at_chunks = {}
            AW = max(n for _, n in ACHUNKS)

            def load_adj_chunk(ci):
                base, n = ACHUNKS[ci]
                a_ = apool.tile(
                    [128, AW * MSH_P], mm_dt, tag="atc", name=f"atc{ci}"
                )
                b.sync.dma_start(
                    out=a_[:, : n * MSH_P],
                    in_=adjT.ap()[:, base * MSH_P : (base + n) * MSH_P],
                )
                at_chunks[ci] = a_

            load_embx(0)
            load_adj_chunk(0)
            load_embx(1)
            load_adj_chunk(1)
            load_embx(2)
            b.sync.dma_start(out=embn_sb[:], in_=embn.ap()[:, :])

            ci_of = {}
            for ci, (base, n) in enumerate(ACHUNKS):
                for t in range(base, base + n):
                    ci_of[t] = (ci, t - base)

            for t in range(KT):
                ci, local = ci_of[t]
                if local == 0 and ci + 2 < len(ACHUNKS) and ci + 2 not in at_chunks:
                    load_adj_chunk(ci + 2)
                at = at_chunks[ci][:, local * MSH_P : (local + 1) * MSH_P]
                et = embx_sb[:, D * t : D * (t + 1)]
                c = t % NDEG
                if t < NDEG:
                    b.vector.tensor_copy(deg_p[c][:], at)
                else:
                    b.vector.tensor_add(deg_p[c][:], deg_p[c][:], at)
                st, sp = (t == 0), (t == KT - 1)
                for j in range(MB):
                    b.tensor.matmul(
                        ps_y[j // 2][:, D * (j % 2) : D * (j % 2) + D],
                        lhsT=at[:, 128 * j : 128 * (j + 1)],
                        rhs=et,
                        start=st,
                        stop=sp,
                    )

            # per-node degree: transpose-reduce the DVE partial chains with
            # N=1 matmuls accumulating in PSUM
            for j in range(MB):
                for c in range(NDEG):
                    b.tensor.matmul(
                        deg_ps[:, j : j + 1],
                        lhsT=deg_p[c][:, 128 * j : 128 * (j + 1)],
                        rhs=ones_col[:, :1],
                        start=(c == 0),
                        stop=(c == NDEG - 1),
                    )
            deg_sb = wpool.tile([128, MB], F32, tag="degsb", bufs=1)
            b.scalar.activation(deg_sb[:], deg_ps[:], AF.Copy, bias=1e-6)
            for j in range(MB):
                t1 = wpool.tile([128, D], mm_dt, tag="t1")
                b.vector.tensor_scalar_mul(
                    t1[:], embn_sb[:, D * j : D * (j + 1)], deg_sb[:, j : j + 1]
                )
                xp = wpool.tile([128, D], mm_dt, tag="xp")
                b.vector.tensor_add(xp[:], t1[:], ps_y[j // 2][:, D * (j % 2) : D * (j % 2) + D])
                sq = wpool.tile([128, D], mm_dt, tag="sq")
                ns = wpool.tile([128, 1], F32, tag="ns")
                b.vector.scalar_tensor_tensor(
                    sq[:], xp[:], 1.0, xp[:], OP.mult, OP.mult, accum_out=ns[:, :1]
                )
                r2 = wpool.tile([128, 1], F32, tag="r2")
                b.vector.reciprocal(r2[:, :1], ns[:, :1])
                rn = wpool.tile([128, 1], F32, tag="rn")
                b.scalar.sqrt(rn[:, :1], r2[:, :1])
                xo = wpool.tile([128, D], mm_dt, tag="xo")
                b.vector.tensor_scalar_mul(xo[:], xp[:], rn[:, :1])
                b.sync.dma_start(out=xn.ap()[128 * j : 128 * (j + 1), :], in_=xo[:])
    b.compile()
    return b


def build_stage2(dat_dt=BF16, mask_dt=BF16):
    """Per-core: w [1, 512] from pre-tiled transposed mask tables + xnT."""
    b = bacc.Bacc(
        "TRN2",
        target_bir_lowering=False,
        debug=False,
        num_devices=NC,
        dynamic_dma_scratch_size=65536,
    )
    xnt = b.dram_tensor("xnt", [D, NP_PAD], dat_dt, kind="ExternalInput")
    uv = b.dram_tensor("uv", [D, 2 * QL], dat_dt, kind="ExternalInput")
    est = b.dram_tensor("est", [128, CT * QL], mask_dt, kind="ExternalInput")
    edt = b.dram_tensor("edt", [128, CT * QL], mask_dt, kind="ExternalInput")
    w = b.dram_tensor("w", [1, QL], F32, kind="ExternalOutput")

    XC = 1264  # xnt resident-load column chunk

    with tile.TileContext(b) as tc:
        with (
            tc.tile_pool(name="const", bufs=1) as cpool,
            tc.tile_pool(name="mask", bufs=3) as mpool,
            tc.tile_pool(name="mid", bufs=3) as spool,
            tc.tile_pool(name="cos", bufs=3, space="PSUM") as ppool,
            tc.tile_pool(name="acc", bufs=1, space="PSUM") as qpool,
        ):
            ones_col = cpool.tile([128, 1], dat_dt)
            b.vector.memset(ones_col[:, :1], 1.0)
            uv_sb = []
            for i in range(2):
                u_ = cpool.tile([128, 2 * QL], dat_dt, tag=f"uv{i}", name=f"uv{i}")
                b.sync.dma_start(out=u_[:], in_=uv.ap()[128 * i : 128 * (i + 1), :])
                uv_sb.append(u_)
            xnt_sb = [
                cpool.tile([128, NP_PAD], dat_dt, tag=f"xnt{i}", name=f"xnt{i}")
                for i in range(2)
            ]

            def load_xnt_chunk(ci):
                c0 = ci * XC
                cw = min(XC, NP_PAD - c0)
                if cw <= 0:
                    return
                for i in range(2):
                    b.sync.dma_start(
                        out=xnt_sb[i][:, c0 : c0 + cw],
                        in_=xnt.ap()[128 * i : 128 * (i + 1), c0 : c0 + cw],
                    )

            MCHUNKS = _chunk_plan(CT, 4, 8)
            cn_tiles = [None] * len(MCHUNKS)
            MW = max(n for _, n in MCHUNKS)

            def load_mask_chunk(ch):
                base, n = MCHUNKS[ch]
                c0, cw = base * QL, n * QL
                m_ = mpool.tile([128, MW * QL], mask_dt, tag="cn", name=f"cn{ch}")
                b.sync.dma_start(out=m_[:, :cw], in_=est.ap()[:, c0 : c0 + cw])
                # src+dst mask sum computed inline by the DMA (CCE add);
                # (sum > 1.5) recovers the AND in the fused STT below.
                # CCE tops out at 2048 elements per descriptor - slice.
                for a0 in range(0, cw, 2048):
                    aw = min(2048, cw - a0)
                    b.gpsimd.dma_start(
                        out=m_[:, a0 : a0 + aw],
                        in_=edt.ap()[:, c0 + a0 : c0 + a0 + aw],
                        accum_op=OP.add,
                    )
                cn_tiles[ch] = m_

            ch_of = {}
            for ch, (base, n) in enumerate(MCHUNKS):
                for t in range(base, base + n):
                    ch_of[t] = (ch, t - base)

            # issue order: uv, first xnt chunk, first mask chunks, rest of xnt
            load_xnt_chunk(0)
            load_mask_chunk(0)
            load_xnt_chunk(1)
            load_mask_chunk(1)
            for ci in range(2, (NP_PAD + XC - 1) // XC):
                load_xnt_chunk(ci)

            ps_w = qpool.tile([1, QL], F32, tag="psw")
            for ct in range(CT):
                ch, local = ch_of[ct]
                off = local * QL
                if local == 0 and ch + 2 < len(MCHUNKS) and cn_tiles[ch + 2] is None:
                    load_mask_chunk(ch + 2)
                csl = slice(128 * ct, 128 * (ct + 1))
                psL = ppool.tile([128, QL], F32, tag="psL")
                psR = ppool.tile([128, QL], F32, tag="psR")
                b.tensor.matmul(psL[:], lhsT=xnt_sb[0][:, csl], rhs=uv_sb[0][:, :QL],
                                start=True, stop=False)
                b.tensor.matmul(psR[:], lhsT=xnt_sb[0][:, csl], rhs=uv_sb[0][:, QL:],
                                start=True, stop=False)
                b.tensor.matmul(psL[:], lhsT=xnt_sb[1][:, csl], rhs=uv_sb[1][:, :QL],
                                start=False, stop=True)
                b.tensor.matmul(psR[:], lhsT=xnt_sb[1][:, csl], rhs=uv_sb[1][:, QL:],
                                start=False, stop=True)
                cos_sb = spool.tile([128, QL], BF16, tag="cossb")
                b.scalar.copy(cos_sb[:], psL[:])
                m1 = spool.tile([128, QL], BF16, tag="m1")
                b.vector.scalar_tensor_tensor(
                    m1[:], cn_tiles[ch][:, off : off + QL], 1.5, psR[:],
                    OP.is_gt, OP.mult,
                )
                m2 = spool.tile([128, QL], BF16, tag="m2")
                b.vector.tensor_mul(m2[:], m1[:], cos_sb[:])
                b.tensor.matmul(ps_w[:1, :], lhsT=ones_col[:, :1], rhs=m2[:],
                                start=(ct == 0), stop=(ct == CT - 1))
            sg = spool.tile([1, QL], F32, tag="sg")
            b.scalar.activation(sg[:1, :], ps_w[:1, :], AF.Sigmoid)
            b.sync.dma_start(out=w.ap()[:1, :], in_=sg[:1, :])
    b.compile()
    return b


def _ktile_pack(x_pad, width):
    # [KT*128, width] -> [128, KT*width] with k-tile t at cols [t*width, (t+1)*width)
    kt = x_pad.shape[0] // 128
    return np.ascontiguousarray(
        x_pad.reshape(kt, 128, width).transpose(1, 0, 2).reshape(128, kt * width)
    )


def make_stage1_inputs(emb, adj_bf):
    e_pad = np.zeros((KP, D), NP_BF16)
    e_pad[:N] = emb
    embx = _ktile_pack(e_pad, D)
    adjT_all = np.ascontiguousarray(adj_bf.T)  # [N, N]: [src k, node m]
    ins = []
    for k in range(NC):
        adjT = np.zeros((KP, MSH_P), NP_BF16)
        adjT[:N, :MSH] = adjT_all[:, k * MSH : (k + 1) * MSH]
        embn = np.ones((MSH_P, D), NP_BF16)
        embn[:MSH] = emb[k * MSH : (k + 1) * MSH]
        ins.append({
            "adjT": _ktile_pack(adjT, MSH_P),
            "embx": embx,
            "embn": _ktile_pack(embn, D),
        })
    return ins


def make_stage2_inputs(adj_bf, xnt_pad, src, dst_):
    ins = []
    for k in range(NC):
        s_k = src[k * QL : (k + 1) * QL]
        d_k = dst_[k * QL : (k + 1) * QL]
        uv = np.concatenate([xnt_pad[:, s_k], xnt_pad[:, d_k]], axis=1)

        def tilemask(idx):
            Bp = np.zeros((QL, NP_PAD), NP_BF16)
            Bp[:, :N] = adj_bf[idx]
            return np.ascontiguousarray(
                Bp.reshape(QL, CT, 128).transpose(2, 1, 0).reshape(128, CT * QL)
            )

        ins.append(
            {
                "xnt": xnt_pad,
                "uv": np.ascontiguousarray(uv),
                "est": tilemask(s_k),
                "edt": tilemask(d_k),
            }
        )
    return ins


_progs = {}
LAST_RESULTS = []  # BassKernelResults of the most recent kernel() call (for profiling)


def _get(name, builder):
    if name not in _progs:
        _progs[name] = builder()
    return _progs[name]


def kernel(emb_weight, adj, edges):
    emb = np.asarray(emb_weight, dtype=np.float32)
    adj = np.asarray(adj, dtype=np.float32)
    edges = np.asarray(edges)
    src = edges[0].astype(np.int64)
    dst_ = edges[1].astype(np.int64)
    adj_bf = adj.astype(NP_BF16)

    s1 = _get("s1", build_stage1)
    s2 = _get("s2", build_stage2)

    in1 = make_stage1_inputs(emb, adj_bf)
    r1 = bass_utils.run_bass_kernel_spmd(s1, in1, core_ids=list(range(NC)))
    xn_full = np.concatenate(
        [np.asarray(r1.results[k]["xn"])[:MSH] for k in range(NC)], axis=0
    )  # [N, D] bf16, node-major
    xnt_pad = np.zeros((D, NP_PAD), NP_BF16)
    xnt_pad[:, :N] = xn_full.T

    in2 = make_stage2_inputs(adj_bf, xnt_pad, src, dst_)
    r2 = bass_utils.run_bass_kernel_spmd(s2, in2, core_ids=list(range(NC)))
    w = np.concatenate([np.asarray(r2.results[k]["w"])[0] for k in range(NC)])

    LAST_RESULTS.clear()
    LAST_RESULTS.extend([r1, r2])
    return w.astype(np.float32)


# revision 12
# speedup vs baseline: 1.2564x; 1.2564x over previous
"""CommonNeighborsPredictor kernel for 8 Trainium2 NeuronCores.

Math (see reference):
    deg = adj.sum(-1) + 1e-6
    x   = emb + (adj @ emb) / deg[:, None]
    xn  = x / max(||x||_2, 1e-8)                            # row-normalize
    w_e = sum_c adj[src_e, c] * adj[dst_e, c] * (xn[src_e]@xn[c]) * (xn[dst_e]@xn[c])
    out = sigmoid(w)

Distribution (2 SPMD launches, no collectives):

  Stage 1 (node-major): shard nodes 8 ways; core k computes xn for its 1250
    nodes.  The k-loop streams adjT tiles [128k, 1280m] (one wide DMA each)
    and emb k-slices [128k, 256]; the PE runs 10 matmuls per k-tile with the
    adjT slice as the stationary operand, accumulating y = adj@emb in
    node-major PSUM ([128 nodes x 256 dims], two blocks packed per bank).
    Degrees accumulate on DVE (exact 0/1 sums in bf16) and are reduced
    per-node with tiny N=1 transpose-matmuls.  The epilogue uses the
    scale-invariance of cosine: x' = deg*emb + y (no division), per-node
    scalars live in [128,1] columns (fast DVE reciprocal + ACT sqrt), and
    scale application is a 4x-mode DVE tensor_scalar.  Host transposes the
    node-major xn shards into xnT.

  Stage 2 (candidate-major): shard query edges 8 ways (512 each).  The host
    lays out per-edge adjacency tables TRANSPOSED and pre-tiled
    (est[p, 512*ct + e] = adj[src_e, 128*ct+p]) so the kernel does plain
    sequential DMA - no indirect gathers, no gpsimd descriptor storms.  The
    src*dst mask product cn = min(aS, aD) is computed INLINE in the DMA
    (CCE min accumulate on the SWDGE path) - no compute engine touches it.
    Per candidate tile [128c x 512e]: PE matmuls produce cosL/cosR against
    resident xnT slices (stationary) and ut|vt (moving), ACT copies the
    PSUM to bf16 SBUF, DVE does the two mask/cos products at 2x bf16 rate,
    and a ones-vector matmul accumulates the candidate-dim reduction across
    all 79 tiles into a single [1, 512] PSUM row.  Sigmoid on ACT.

dtypes: adjacency and matmul operands bf16 (adjacency 0/1 exact; emb/xn
rounding contributes ~1e-4 output error vs the fp32 reference).  PSUM and
per-node scalars fp32.
"""

import numpy as np

import concourse.bass as bass
import concourse.bacc as bacc
import concourse.mybir as mybir
import concourse.tile as tile
from concourse import bass_utils

F32 = mybir.dt.float32
BF16 = mybir.dt.bfloat16
FP8 = mybir.dt.float8e4
AF = mybir.ActivationFunctionType
OP = mybir.AluOpType
NP_BF16 = mybir.dt.np(BF16)

N, D, Q, NC = 10000, 256, 4096, 8
KT = 79                  # contraction tiles over source nodes (N padded)
KP = KT * 128            # 10112
MSH = N // NC            # 1250 nodes per core
MB = 10                  # node blocks per core
MSH_P = MB * 128         # 1280 (padded shard)
QL = Q // NC             # 512 edges per core
CT = 79                  # candidate tiles in stage 2
NP_PAD = CT * 128        # 10112
CH = 8                   # candidate tiles per mask DMA chunk
NCH = (CT + CH - 1) // CH


def _chunk_plan(total, first, step):
    out, base = [], 0
    n = first
    while base < total:
        n = min(n, total - base)
        out.append((base, n))
        base += n
        n = step
    return out


def build_stage1(mm_dt=BF16):
    """Per-core: xn [1280, 256] node-major from adjT shard + emb."""
    b = bacc.Bacc("TRN2", target_bir_lowering=False, debug=False, num_devices=NC)
    adjT = b.dram_tensor("adjT", [128, KT * MSH_P], mm_dt, kind="ExternalInput")
    embx = b.dram_tensor("embx", [128, KT * D], mm_dt, kind="ExternalInput")
    embn = b.dram_tensor("embn", [128, MB * D], mm_dt, kind="ExternalInput")
    xn = b.dram_tensor("xn", [MSH_P, D], mm_dt, kind="ExternalOutput")

    NDEG = 2
    ACHUNKS = _chunk_plan(KT, 2, 8)
    with tile.TileContext(b) as tc:
        with (
            tc.tile_pool(name="const", bufs=1) as cpool,
            tc.tile_pool(name="adjs", bufs=3) as apool,
            tc.tile_pool(name="work", bufs=3) as wpool,
            tc.tile_pool(name="py", bufs=1, space="PSUM") as ypool,
            tc.tile_pool(name="pd", bufs=1, space="PSUM") as dpool,
        ):
            ones_col = cpool.tile([128, 1], mm_dt)
            b.vector.memset(ones_col[:, :1], 1.0)
            deg_p = [
                cpool.tile([128, MSH_P], mm_dt, tag=f"degp{c}", name=f"degp{c}")
                for c in range(NDEG)
            ]
            ps_y = [
                ypool.tile([128, 2 * D], F32, tag=f"py{h}", name=f"py{h}")
                for h in range(MB // 2)
            ]
            deg_ps = dpool.tile([128, MB], F32, tag="degps")

            # resident emb (k-major); chunk loads interleaved 1:1 with the
            # adjT chunks covering the same k-range so neither starves the PE
            embx_sb = cpool.tile([128, KT * D], mm_dt, tag="embx")

            def load_embx_chunk(ci):
                base, n = ACHUNKS[ci]
                b.sync.dma_start(
                    out=embx_sb[:, base * D : (base + n) * D],
                    in_=embx.ap()[:, base * D : (base + n) * D],
                )

            embn_sb = cpool.tile([128, MB * D], mm_dt, tag="embn")
            at_chunks = {}
            AW = max(n for _, n in ACHUNKS)

            def load_adj_chunk(ci):
                base, n = ACHUNKS[ci]
                a_ = apool.tile(
                    [128, AW * MSH_P], mm_dt, tag="atc", name=f"atc{ci}"
                )
                b.sync.dma_start(
                    out=a_[:, : n * MSH_P],
                    in_=adjT.ap()[:, base * MSH_P : (base + n) * MSH_P],
                )
                at_chunks[ci] = a_

            for ci in (0, 1):
                load_embx_chunk(ci)
                load_adj_chunk(ci)

            ci_of = {}
            for ci, (base, n) in enumerate(ACHUNKS):
                for t in range(base, base + n):
                    ci_of[t] = (ci, t - base)

            for t in range(KT):
                ci, local = ci_of[t]
                if local == 0 and ci + 2 < len(ACHUNKS) and ci + 2 not in at_chunks:
                    load_embx_chunk(ci + 2)
                    load_adj_chunk(ci + 2)
                if t == 40:
                    b.sync.dma_start(out=embn_sb[:], in_=embn.ap()[:, :])
                at = at_chunks[ci][:, local * MSH_P : (local + 1) * MSH_P]
                et = embx_sb[:, D * t : D * (t + 1)]
                c = t % NDEG
                if t < NDEG:
                    b.vector.tensor_copy(deg_p[c][:], at)
                else:
                    b.vector.tensor_add(deg_p[c][:], deg_p[c][:], at)
                st, sp = (t == 0), (t == KT - 1)
                for j in range(MB):
                    b.tensor.matmul(
                        ps_y[j // 2][:, D * (j % 2) : D * (j % 2) + D],
                        lhsT=at[:, 128 * j : 128 * (j + 1)],
                        rhs=et,
                        start=st,
                        stop=sp,
                    )

            # per-node degree: transpose-reduce the DVE partial chains with
            # N=1 matmuls accumulating in PSUM
            for j in range(MB):
                for c in range(NDEG):
                    b.tensor.matmul(
                        deg_ps[:, j : j + 1],
                        lhsT=deg_p[c][:, 128 * j : 128 * (j + 1)],
                        rhs=ones_col[:, :1],
                        start=(c == 0),
                        stop=(c == NDEG - 1),
                    )
            deg_sb = wpool.tile([128, MB], F32, tag="degsb", bufs=1)
            b.scalar.activation(deg_sb[:], deg_ps[:], AF.Copy, bias=1e-6)
            for j in range(MB):
                t1 = wpool.tile([128, D], mm_dt, tag="t1")
                b.vector.tensor_scalar_mul(
                    t1[:], embn_sb[:, D * j : D * (j + 1)], deg_sb[:, j : j + 1]
                )
                xp = wpool.tile([128, D], mm_dt, tag="xp")
                b.vector.tensor_add(xp[:], t1[:], ps_y[j // 2][:, D * (j % 2) : D * (j % 2) + D])
                sq = wpool.tile([128, D], mm_dt, tag="sq")
                ns = wpool.tile([128, 1], F32, tag="ns")
                b.vector.scalar_tensor_tensor(
                    sq[:], xp[:], 1.0, xp[:], OP.mult, OP.mult, accum_out=ns[:, :1]
                )
                r2 = wpool.tile([128, 1], F32, tag="r2")
                b.vector.reciprocal(r2[:, :1], ns[:, :1])
                rn = wpool.tile([128, 1], F32, tag="rn")
                b.scalar.sqrt(rn[:, :1], r2[:, :1])
                xo = wpool.tile([128, D], mm_dt, tag="xo")
                b.vector.tensor_scalar_mul(xo[:], xp[:], rn[:, :1])
                b.sync.dma_start(out=xn.ap()[128 * j : 128 * (j + 1), :], in_=xo[:])
    b.compile()
    return b


def build_stage2(mask_dt=BF16):
    """Per-core: w [1, 512] from pre-tiled transposed mask tables + fp8 xnT.

    cos matmuls run fp8 DoubleRow (K=256 in one MM, lhsT reused by the L and
    R products); candidate tiles are processed in pairs so the ACT PSUM
    eviction, the DVE m2 product, and the ones-matmul reduction each cover
    [128, 1024] per instruction.
    """
    b = bacc.Bacc(
        "TRN2",
        target_bir_lowering=False,
        debug=False,
        num_devices=NC,
        dynamic_dma_scratch_size=65536,
    )
    xnt = b.dram_tensor("xnt", [D, NP_PAD], FP8, kind="ExternalInput")
    uv = b.dram_tensor("uv", [D, 2 * QL], FP8, kind="ExternalInput")
    est = b.dram_tensor("est", [128, CT * QL], mask_dt, kind="ExternalInput")
    edt = b.dram_tensor("edt", [128, CT * QL], mask_dt, kind="ExternalInput")
    w = b.dram_tensor("w", [1, QL], F32, kind="ExternalOutput")

    XC = 1264  # xnt resident-load column chunk
    PM = mybir.MatmulPerfMode

    with tile.TileContext(b) as tc:
        with (
            tc.tile_pool(name="const", bufs=1) as cpool,
            tc.tile_pool(name="mask", bufs=3) as mpool,
            tc.tile_pool(name="mid", bufs=3) as spool,
            tc.tile_pool(name="cosL", bufs=2, space="PSUM") as ppool,
            tc.tile_pool(name="cosR", bufs=2, space="PSUM") as rpool,
            tc.tile_pool(name="acc", bufs=1, space="PSUM") as qpool,
        ):
            ones_col = cpool.tile([128, 1], BF16)
            b.vector.memset(ones_col[:, :1], 1.0)
            uv8 = cpool.tile([128, 2, 2 * QL], FP8, tag="uv8")
            for i in range(2):
                b.sync.dma_start(
                    out=uv8[:, i, :], in_=uv.ap()[128 * i : 128 * (i + 1), :]
                )
            xnt8 = cpool.tile([128, 2, NP_PAD], FP8, tag="xnt8")

            def load_xnt_chunk(ci):
                c0 = ci * XC
                cw = min(XC, NP_PAD - c0)
                if cw <= 0:
                    return
                for i in range(2):
                    b.sync.dma_start(
                        out=xnt8[:, i, c0 : c0 + cw],
                        in_=xnt.ap()[128 * i : 128 * (i + 1), c0 : c0 + cw],
                    )

            MCHUNKS = _chunk_plan(CT, 4, 8)
            cn_tiles = [None] * len(MCHUNKS)
            MW = max(n for _, n in MCHUNKS)

            def load_mask_chunk(ch):
                base, n = MCHUNKS[ch]
                c0, cw = base * QL, n * QL
                m_ = mpool.tile([128, MW * QL], mask_dt, tag="cn", name=f"cn{ch}")
                b.sync.dma_start(out=m_[:, :cw], in_=est.ap()[:, c0 : c0 + cw])
                # src+dst mask sum via inline CCE add (<=2048 elems per DMA)
                for a0 in range(0, cw, 2048):
                    aw = min(2048, cw - a0)
                    b.gpsimd.dma_start(
                        out=m_[:, a0 : a0 + aw],
                        in_=edt.ap()[:, c0 + a0 : c0 + a0 + aw],
                        accum_op=OP.add,
                    )
                cn_tiles[ch] = m_

            ch_of = {}
            for ch, (base, n) in enumerate(MCHUNKS):
                for t in range(base, base + n):
                    ch_of[t] = (ch, t - base)

            load_xnt_chunk(0)
            load_mask_chunk(0)
            load_xnt_chunk(1)
            load_mask_chunk(1)
            for ci in range(2, (NP_PAD + XC - 1) // XC):
                load_xnt_chunk(ci)

            ps_w = qpool.tile([1, QL], F32, tag="psw")
            pairs = [(p, min(2, CT - p)) for p in range(0, CT, 2)]
            for pi, (p0, np_) in enumerate(pairs):
                psL = ppool.tile([128, 2 * QL], F32, tag="psL")
                psRs = []
                for u in range(np_):
                    ct = p0 + u
                    ch, local = ch_of[ct]
                    if local == 0 and ch + 2 < len(MCHUNKS) and cn_tiles[ch + 2] is None:
                        load_mask_chunk(ch + 2)
                    lhs = xnt8[:, :, 128 * ct : 128 * (ct + 1)]
                    b.tensor.matmul(
                        psL[:, QL * u : QL * (u + 1)], lhsT=lhs,
                        rhs=uv8[:, :, :QL], start=True, stop=True,
                        perf_mode=PM.DoubleRow,
                    )
                    pr = rpool.tile([128, QL], F32, tag="psR")
                    b.tensor.matmul(
                        pr[:], lhsT=lhs, rhs=uv8[:, :, QL:],
                        start=True, stop=True, perf_mode=PM.DoubleRow,
                    )
                    psRs.append(pr)
                cos_sb = spool.tile([128, 2 * QL], BF16, tag="cossb")
                b.scalar.copy(cos_sb[:, : QL * np_], psL[:, : QL * np_])
                m1p = spool.tile([128, 2 * QL], BF16, tag="m1p")
                for u in range(np_):
                    ct = p0 + u
                    ch, local = ch_of[ct]
                    b.vector.scalar_tensor_tensor(
                        m1p[:, QL * u : QL * (u + 1)],
                        cn_tiles[ch][:, local * QL : (local + 1) * QL],
                        1.5, psRs[u][:], OP.is_gt, OP.mult,
                    )
                m2p = spool.tile([128, 2 * QL], BF16, tag="m2p")
                b.vector.tensor_mul(
                    m2p[:, : QL * np_], m1p[:, : QL * np_], cos_sb[:, : QL * np_]
                )
                for u in range(np_):
                    b.tensor.matmul(
                        ps_w[:1, :], lhsT=ones_col[:, :1],
                        rhs=m2p[:, QL * u : QL * (u + 1)],
                        start=(pi == 0 and u == 0),
                        stop=(pi == len(pairs) - 1 and u == np_ - 1),
                    )
            sg = spool.tile([1, QL], F32, tag="sg")
            b.scalar.activation(sg[:1, :], ps_w[:1, :], AF.Sigmoid)
            b.sync.dma_start(out=w.ap()[:1, :], in_=sg[:1, :])
    b.compile()
    return b


def _ktile_pack(x_pad, width):
    # [KT*128, width] -> [128, KT*width] with k-tile t at cols [t*width, (t+1)*width)
    kt = x_pad.shape[0] // 128
    return np.ascontiguousarray(
        x_pad.reshape(kt, 128, width).transpose(1, 0, 2).reshape(128, kt * width)
    )


def make_stage1_inputs(emb, adj_bf):
    e_pad = np.zeros((KP, D), NP_BF16)
    e_pad[:N] = emb
    embx = _ktile_pack(e_pad, D)
    adjT_all = np.ascontiguousarray(adj_bf.T)  # [N, N]: [src k, node m]
    ins = []
    for k in range(NC):
        adjT = np.zeros((KP, MSH_P), NP_BF16)
        adjT[:N, :MSH] = adjT_all[:, k * MSH : (k + 1) * MSH]
        embn = np.ones((MSH_P, D), NP_BF16)
        embn[:MSH] = emb[k * MSH : (k + 1) * MSH]
        ins.append({
            "adjT": _ktile_pack(adjT, MSH_P),
            "embx": embx,
            "embn": _ktile_pack(embn, D),
        })
    return ins


NP_FP8 = mybir.dt.np(FP8)


def make_stage2_inputs(adj_bf, xnt_pad, src, dst_):
    xnt8 = xnt_pad.astype(NP_FP8)
    ins = []
    for k in range(NC):
        s_k = src[k * QL : (k + 1) * QL]
        d_k = dst_[k * QL : (k + 1) * QL]
        uv = np.concatenate([xnt8[:, s_k], xnt8[:, d_k]], axis=1)

        def tilemask(idx):
            Bp = np.zeros((QL, NP_PAD), NP_BF16)
            Bp[:, :N] = adj_bf[idx]
            return np.ascontiguousarray(
                Bp.reshape(QL, CT, 128).transpose(2, 1, 0).reshape(128, CT * QL)
            )

        ins.append(
            {
                "xnt": xnt8,
                "uv": np.ascontiguousarray(uv),
                "est": tilemask(s_k),
                "edt": tilemask(d_k),
            }
        )
    return ins


_progs = {}
LAST_RESULTS = []  # BassKernelResults of the most recent kernel() call (for profiling)


def _get(name, builder):
    if name not in _progs:
        _progs[name] = builder()
    return _progs[name]


def kernel(emb_weight, adj, edges):
    emb = np.asarray(emb_weight, dtype=np.float32)
    adj = np.asarray(adj, dtype=np.float32)
    edges = np.asarray(edges)
    src = edges[0].astype(np.int64)
    dst_ = edges[1].astype(np.int64)
    adj_bf = adj.astype(NP_BF16)

    s1 = _get("s1", build_stage1)
    s2 = _get("s2", build_stage2)

    in1 = make_stage1_inputs(emb, adj_bf)
    r1 = bass_utils.run_bass_kernel_spmd(s1, in1, core_ids=list(range(NC)))
    xn_full = np.concatenate(
        [np.asarray(r1.results[k]["xn"])[:MSH] for k in range(NC)], axis=0
    )  # [N, D] bf16, node-major
    xnt_pad = np.zeros((D, NP_PAD), NP_BF16)
    xnt_pad[:, :N] = xn_full.T

    in2 = make_stage2_inputs(adj_bf, xnt_pad, src, dst_)
    r2 = bass_utils.run_bass_kernel_spmd(s2, in2, core_ids=list(range(NC)))
    w = np.concatenate([np.asarray(r2.results[k]["w"])[0] for k in range(NC)])

    LAST_RESULTS.clear()
    LAST_RESULTS.extend([r1, r2])
    return w.astype(np.float32)


# revision 14
# speedup vs baseline: 1.3185x; 1.0494x over previous
"""CommonNeighborsPredictor kernel for 8 Trainium2 NeuronCores.

Math (see reference):
    deg = adj.sum(-1) + 1e-6
    x   = emb + (adj @ emb) / deg[:, None]
    xn  = x / max(||x||_2, 1e-8)                            # row-normalize
    w_e = sum_c adj[src_e, c] * adj[dst_e, c] * (xn[src_e]@xn[c]) * (xn[dst_e]@xn[c])
    out = sigmoid(w)

Distribution (2 SPMD launches, no collectives):

  Stage 1 (node-major): shard nodes 8 ways; core k computes xn for its 1250
    nodes.  adjT comes in host-pre-tiled ([128, 79*1280]) and streams in
    multi-k-tile chunks (big DMA descriptors), interleaved 1:1 with the emb
    k-slices covering the same range so neither starves the PE.  Per k-tile
    the PE runs 10 matmuls with the adjT slice stationary, accumulating
    y = adj@emb in node-major PSUM (two 256-col blocks packed per bank).
    Degrees accumulate on DVE (0/1 sums exact in bf16) and are reduced
    per-node with N=1 transpose-matmuls.  The epilogue exploits cosine
    scale-invariance: x' = deg*emb + y (no division); per-node scalars are
    [128,1] columns (fast DVE reciprocal + ACT sqrt); squares+norms fused in
    one DVE scalar_tensor_tensor with accum_out.  Host transposes xn.

  Stage 2 (candidate-major): shard query edges 8 ways (512 each).  The host
    lays out per-edge adjacency tables TRANSPOSED and pre-tiled
    (est[p, 512*ct + e] = adj[src_e, 128*ct+p], fp8 - 0/1 exact) so the
    kernel does plain sequential DMA - no indirect gathers.  The src+dst
    mask sum is computed INLINE in the DMA (CCE add, <=2048 elems per
    descriptor); (sum > 1.5) recovers the AND inside a fused DVE
    scalar_tensor_tensor against cosR straight from PSUM.  cos matmuls run
    fp8 DoubleRow (K=256 in one MM; xn/ut/vt stored as [128, 2, *] tiles),
    candidate tiles processed in pairs so the ACT PSUM eviction and the DVE
    m2 product cover [128, 1024] per instruction.  A ones-vector matmul
    accumulates the candidate reduction across all 79 tiles into one
    [1, 512] PSUM row; sigmoid on ACT.

dtypes: adjacency bf16/fp8 (0/1 exact), stage-1 matmuls bf16, stage-2 cos
matmuls fp8e4 (DoubleRow), elementwise bf16, PSUM and per-node scalars
fp32.  Measured rel err vs fp32 reference ~8e-4.
"""

import numpy as np

import concourse.bass as bass
import concourse.bacc as bacc
import concourse.mybir as mybir
import concourse.tile as tile
from concourse import bass_utils

F32 = mybir.dt.float32
BF16 = mybir.dt.bfloat16
FP8 = mybir.dt.float8e4
AF = mybir.ActivationFunctionType
OP = mybir.AluOpType
NP_BF16 = mybir.dt.np(BF16)

N, D, Q, NC = 10000, 256, 4096, 8
KT = 79                  # contraction tiles over source nodes (N padded)
KP = KT * 128            # 10112
MSH = N // NC            # 1250 nodes per core
MB = 10                  # node blocks per core
MSH_P = MB * 128         # 1280 (padded shard)
QL = Q // NC             # 512 edges per core
CT = 79                  # candidate tiles in stage 2
NP_PAD = CT * 128        # 10112
CH = 8                   # candidate tiles per mask DMA chunk
NCH = (CT + CH - 1) // CH


def _chunk_plan(total, first, step):
    out, base = [], 0
    n = first
    while base < total:
        n = min(n, total - base)
        out.append((base, n))
        base += n
        n = step
    return out


def build_stage1(mm_dt=BF16):
    """Per-core: xn [1280, 256] node-major from adjT shard + emb."""
    b = bacc.Bacc("TRN2", target_bir_lowering=False, debug=False, num_devices=NC)
    adjT = b.dram_tensor("adjT", [128, KT * MSH_P], mm_dt, kind="ExternalInput")
    embx = b.dram_tensor("embx", [128, KT * D], mm_dt, kind="ExternalInput")
    embn = b.dram_tensor("embn", [128, MB * D], mm_dt, kind="ExternalInput")
    xn = b.dram_tensor("xn", [MSH_P, D], mm_dt, kind="ExternalOutput")

    NDEG = 2
    ACHUNKS = [(0, 2), (2, 4)] + [(6 + b, n) for b, n in _chunk_plan(KT - 6, 8, 8)]
    with tile.TileContext(b) as tc:
        with (
            tc.tile_pool(name="const", bufs=1) as cpool,
            tc.tile_pool(name="adjs", bufs=3) as apool,
            tc.tile_pool(name="work", bufs=3) as wpool,
            tc.tile_pool(name="py", bufs=1, space="PSUM") as ypool,
            tc.tile_pool(name="pd", bufs=1, space="PSUM") as dpool,
        ):
            ones_col = cpool.tile([128, 1], mm_dt)
            b.vector.memset(ones_col[:, :1], 1.0)
            deg_p = [
                cpool.tile([128, MSH_P], mm_dt, tag=f"degp{c}", name=f"degp{c}")
                for c in range(NDEG)
            ]
            ps_y = [
                ypool.tile([128, 2 * D], F32, tag=f"py{h}", name=f"py{h}")
                for h in range(MB // 2)
            ]
            deg_ps = dpool.tile([128, MB], F32, tag="degps")

            # resident emb (k-major); chunk loads interleaved 1:1 with the
            # adjT chunks covering the same k-range so neither starves the PE
            embx_sb = cpool.tile([128, KT * D], mm_dt, tag="embx")

            def load_embx_chunk(ci):
                base, n = ACHUNKS[ci]
                b.sync.dma_start(
                    out=embx_sb[:, base * D : (base + n) * D],
                    in_=embx.ap()[:, base * D : (base + n) * D],
                )

            embn_sb = cpool.tile([128, MB * D], mm_dt, tag="embn")
            at_chunks = {}
            AW = max(n for _, n in ACHUNKS)

            def load_adj_chunk(ci):
                base, n = ACHUNKS[ci]
                a_ = apool.tile(
                    [128, AW * MSH_P], mm_dt, tag="atc", name=f"atc{ci}"
                )
                b.sync.dma_start(
                    out=a_[:, : n * MSH_P],
                    in_=adjT.ap()[:, base * MSH_P : (base + n) * MSH_P],
                )
                at_chunks[ci] = a_

            for ci in (0, 1):
                load_embx_chunk(ci)
                load_adj_chunk(ci)

            ci_of = {}
            for ci, (base, n) in enumerate(ACHUNKS):
                for t in range(base, base + n):
                    ci_of[t] = (ci, t - base)

            for t in range(KT):
                ci, local = ci_of[t]
                if local == 0 and ci + 2 < len(ACHUNKS) and ci + 2 not in at_chunks:
                    load_embx_chunk(ci + 2)
                    load_adj_chunk(ci + 2)
                if t == 40:
                    b.sync.dma_start(out=embn_sb[:], in_=embn.ap()[:, :])
                at = at_chunks[ci][:, local * MSH_P : (local + 1) * MSH_P]
                et = embx_sb[:, D * t : D * (t + 1)]
                c = t % NDEG
                if t < NDEG:
                    b.vector.tensor_copy(deg_p[c][:], at)
                else:
                    b.vector.tensor_add(deg_p[c][:], deg_p[c][:], at)
                st, sp = (t == 0), (t == KT - 1)
                for j in range(MB):
                    b.tensor.matmul(
                        ps_y[j // 2][:, D * (j % 2) : D * (j % 2) + D],
                        lhsT=at[:, 128 * j : 128 * (j + 1)],
                        rhs=et,
                        start=st,
                        stop=sp,
                    )

            # per-node degree: transpose-reduce the DVE partial chains with
            # N=1 matmuls accumulating in PSUM
            for j in range(MB):
                for c in range(NDEG):
                    b.tensor.matmul(
                        deg_ps[:, j : j + 1],
                        lhsT=deg_p[c][:, 128 * j : 128 * (j + 1)],
                        rhs=ones_col[:, :1],
                        start=(c == 0),
                        stop=(c == NDEG - 1),
                    )
            deg_sb = wpool.tile([128, MB], F32, tag="degsb", bufs=1)
            b.scalar.activation(deg_sb[:], deg_ps[:], AF.Copy, bias=1e-6)
            for j in range(MB):
                t1 = wpool.tile([128, D], mm_dt, tag="t1")
                b.vector.tensor_scalar_mul(
                    t1[:], embn_sb[:, D * j : D * (j + 1)], deg_sb[:, j : j + 1]
                )
                xp = wpool.tile([128, D], mm_dt, tag="xp")
                b.vector.tensor_add(xp[:], t1[:], ps_y[j // 2][:, D * (j % 2) : D * (j % 2) + D])
                sq = wpool.tile([128, D], mm_dt, tag="sq")
                ns = wpool.tile([128, 1], F32, tag="ns")
                b.vector.scalar_tensor_tensor(
                    sq[:], xp[:], 1.0, xp[:], OP.mult, OP.mult, accum_out=ns[:, :1]
                )
                r2 = wpool.tile([128, 1], F32, tag="r2")
                b.vector.reciprocal(r2[:, :1], ns[:, :1])
                rn = wpool.tile([128, 1], F32, tag="rn")
                b.scalar.sqrt(rn[:, :1], r2[:, :1])
                xo = wpool.tile([128, D], mm_dt, tag="xo")
                b.vector.tensor_scalar_mul(xo[:], xp[:], rn[:, :1])
                b.sync.dma_start(out=xn.ap()[128 * j : 128 * (j + 1), :], in_=xo[:])
    b.compile()
    return b


def build_stage2(mask_dt=FP8):
    """Per-core: w [1, 512] from pre-tiled transposed mask tables + fp8 xnT.

    cos matmuls run fp8 DoubleRow (K=256 in one MM, lhsT reused by the L and
    R products); candidate tiles are processed in pairs so the ACT PSUM
    eviction, the DVE m2 product, and the ones-matmul reduction each cover
    [128, 1024] per instruction.
    """
    b = bacc.Bacc(
        "TRN2",
        target_bir_lowering=False,
        debug=False,
        num_devices=NC,
        dynamic_dma_scratch_size=65536,
    )
    xnt = b.dram_tensor("xnt", [D, NP_PAD], FP8, kind="ExternalInput")
    uv = b.dram_tensor("uv", [D, 2 * QL], FP8, kind="ExternalInput")
    est = b.dram_tensor("est", [128, CT * QL], mask_dt, kind="ExternalInput")
    edt = b.dram_tensor("edt", [128, CT * QL], mask_dt, kind="ExternalInput")
    w = b.dram_tensor("w", [1, QL], F32, kind="ExternalOutput")

    XC = 1264  # xnt resident-load column chunk
    PM = mybir.MatmulPerfMode

    with tile.TileContext(b) as tc:
        with (
            tc.tile_pool(name="const", bufs=1) as cpool,
            tc.tile_pool(name="mask", bufs=3) as mpool,
            tc.tile_pool(name="mid", bufs=3) as spool,
            tc.tile_pool(name="cosL", bufs=2, space="PSUM") as ppool,
            tc.tile_pool(name="cosR", bufs=2, space="PSUM") as rpool,
            tc.tile_pool(name="acc", bufs=1, space="PSUM") as qpool,
        ):
            ones_col = cpool.tile([128, 1], BF16)
            b.vector.memset(ones_col[:, :1], 1.0)
            uv8 = cpool.tile([128, 2, 2 * QL], FP8, tag="uv8")
            for i in range(2):
                b.sync.dma_start(
                    out=uv8[:, i, :], in_=uv.ap()[128 * i : 128 * (i + 1), :]
                )
            xnt8 = cpool.tile([128, 2, NP_PAD], FP8, tag="xnt8")

            def load_xnt_chunk(ci):
                c0 = ci * XC
                cw = min(XC, NP_PAD - c0)
                if cw <= 0:
                    return
                for i in range(2):
                    b.sync.dma_start(
                        out=xnt8[:, i, c0 : c0 + cw],
                        in_=xnt.ap()[128 * i : 128 * (i + 1), c0 : c0 + cw],
                    )

            MCHUNKS = _chunk_plan(CT, 4, 8)
            cn_tiles = [None] * len(MCHUNKS)
            MW = max(n for _, n in MCHUNKS)

            def load_mask_chunk(ch):
                base, n = MCHUNKS[ch]
                c0, cw = base * QL, n * QL
                m_ = mpool.tile([128, MW * QL], mask_dt, tag="cn", name=f"cn{ch}")
                b.sync.dma_start(out=m_[:, :cw], in_=est.ap()[:, c0 : c0 + cw])
                # src+dst mask sum via inline CCE add (<=2048 elems per DMA)
                for a0 in range(0, cw, 2048):
                    aw = min(2048, cw - a0)
                    b.gpsimd.dma_start(
                        out=m_[:, a0 : a0 + aw],
                        in_=edt.ap()[:, c0 + a0 : c0 + a0 + aw],
                        accum_op=OP.add,
                    )
                cn_tiles[ch] = m_

            ch_of = {}
            for ch, (base, n) in enumerate(MCHUNKS):
                for t in range(base, base + n):
                    ch_of[t] = (ch, t - base)

            load_xnt_chunk(0)
            load_mask_chunk(0)
            load_xnt_chunk(1)
            load_mask_chunk(1)
            for ci in range(2, (NP_PAD + XC - 1) // XC):
                load_xnt_chunk(ci)

            ps_w = qpool.tile([1, QL], F32, tag="psw")
            pairs = [(p, min(2, CT - p)) for p in range(0, CT, 2)]
            for pi, (p0, np_) in enumerate(pairs):
                psL = ppool.tile([128, 2 * QL], F32, tag="psL")
                psRs = []
                for u in range(np_):
                    ct = p0 + u
                    ch, local = ch_of[ct]
                    if local == 0 and ch + 2 < len(MCHUNKS) and cn_tiles[ch + 2] is None:
                        load_mask_chunk(ch + 2)
                    lhs = xnt8[:, :, 128 * ct : 128 * (ct + 1)]
                    b.tensor.matmul(
                        psL[:, QL * u : QL * (u + 1)], lhsT=lhs,
                        rhs=uv8[:, :, :QL], start=True, stop=True,
                        perf_mode=PM.DoubleRow,
                    )
                    pr = rpool.tile([128, QL], F32, tag="psR")
                    b.tensor.matmul(
                        pr[:], lhsT=lhs, rhs=uv8[:, :, QL:],
                        start=True, stop=True, perf_mode=PM.DoubleRow,
                    )
                    psRs.append(pr)
                cos_sb = spool.tile([128, 2 * QL], BF16, tag="cossb")
                b.scalar.copy(cos_sb[:, : QL * np_], psL[:, : QL * np_])
                m1p = spool.tile([128, 2 * QL], BF16, tag="m1p")
                for u in range(np_):
                    ct = p0 + u
                    ch, local = ch_of[ct]
                    b.vector.scalar_tensor_tensor(
                        m1p[:, QL * u : QL * (u + 1)],
                        cn_tiles[ch][:, local * QL : (local + 1) * QL],
                        1.5, psRs[u][:], OP.is_gt, OP.mult,
                    )
                m2p = spool.tile([128, 2 * QL], BF16, tag="m2p")
                b.vector.tensor_mul(
                    m2p[:, : QL * np_], m1p[:, : QL * np_], cos_sb[:, : QL * np_]
                )
                for u in range(np_):
                    b.tensor.matmul(
                        ps_w[:1, :], lhsT=ones_col[:, :1],
                        rhs=m2p[:, QL * u : QL * (u + 1)],
                        start=(pi == 0 and u == 0),
                        stop=(pi == len(pairs) - 1 and u == np_ - 1),
                    )
            sg = spool.tile([1, QL], F32, tag="sg")
            b.scalar.activation(sg[:1, :], ps_w[:1, :], AF.Sigmoid)
            b.sync.dma_start(out=w.ap()[:1, :], in_=sg[:1, :])
    b.compile()
    return b


def _ktile_pack(x_pad, width):
    # [KT*128, width] -> [128, KT*width] with k-tile t at cols [t*width, (t+1)*width)
    kt = x_pad.shape[0] // 128
    return np.ascontiguousarray(
        x_pad.reshape(kt, 128, width).transpose(1, 0, 2).reshape(128, kt * width)
    )


def make_stage1_inputs(emb, adj_bf):
    e_pad = np.zeros((KP, D), NP_BF16)
    e_pad[:N] = emb
    embx = _ktile_pack(e_pad, D)
    adjT_all = np.ascontiguousarray(adj_bf.T)  # [N, N]: [src k, node m]
    ins = []
    for k in range(NC):
        adjT = np.zeros((KP, MSH_P), NP_BF16)
        adjT[:N, :MSH] = adjT_all[:, k * MSH : (k + 1) * MSH]
        embn = np.ones((MSH_P, D), NP_BF16)
        embn[:MSH] = emb[k * MSH : (k + 1) * MSH]
        ins.append({
            "adjT": _ktile_pack(adjT, MSH_P),
            "embx": embx,
            "embn": _ktile_pack(embn, D),
        })
    return ins


NP_FP8 = mybir.dt.np(FP8)


def make_stage2_inputs(adj_bf, xnt_pad, src, dst_):
    xnt8 = xnt_pad.astype(NP_FP8)
    ins = []
    for k in range(NC):
        s_k = src[k * QL : (k + 1) * QL]
        d_k = dst_[k * QL : (k + 1) * QL]
        uv = np.concatenate([xnt8[:, s_k], xnt8[:, d_k]], axis=1)

        def tilemask(idx):
            Bp = np.zeros((QL, NP_PAD), NP_FP8)
            Bp[:, :N] = adj_bf[idx].astype(NP_FP8)
            return np.ascontiguousarray(
                Bp.reshape(QL, CT, 128).transpose(2, 1, 0).reshape(128, CT * QL)
            )

        ins.append(
            {
                "xnt": xnt8,
                "uv": np.ascontiguousarray(uv),
                "est": tilemask(s_k),
                "edt": tilemask(d_k),
            }
        )
    return ins


_progs = {}
LAST_RESULTS = []  # BassKernelResults of the most recent kernel() call (for profiling)


def _get(name, builder):
    if name not in _progs:
        _progs[name] = builder()
    return _progs[name]


def kernel(emb_weight, adj, edges):
    emb = np.asarray(emb_weight, dtype=np.float32)
    adj = np.asarray(adj, dtype=np.float32)
    edges = np.asarray(edges)
    src = edges[0].astype(np.int64)
    dst_ = edges[1].astype(np.int64)
    adj_bf = adj.astype(NP_BF16)

    s1 = _get("s1", build_stage1)
    s2 = _get("s2", build_stage2)

    in1 = make_stage1_inputs(emb, adj_bf)
    r1 = bass_utils.run_bass_kernel_spmd(s1, in1, core_ids=list(range(NC)))
    xn_full = np.concatenate(
        [np.asarray(r1.results[k]["xn"])[:MSH] for k in range(NC)], axis=0
    )  # [N, D] bf16, node-major
    xnt_pad = np.zeros((D, NP_PAD), NP_BF16)
    xnt_pad[:, :N] = xn_full.T

    in2 = make_stage2_inputs(adj_bf, xnt_pad, src, dst_)
    r2 = bass_utils.run_bass_kernel_spmd(s2, in2, core_ids=list(range(NC)))
    w = np.concatenate([np.asarray(r2.results[k]["w"])[0] for k in range(NC)])

    LAST_RESULTS.clear()
    LAST_RESULTS.extend([r1, r2])
    return w.astype(np.float32)
